# revision 33
# baseline (speedup 1.0000x reference)
"""Trainium2 Bass kernel for nn_BinaryConv2d (B=16, C=64, H=W=256, 3x3, pad 1).

Forward semantics (STE forward values):
  act = sign(x * rd_k + rd_b)                  in {-1, 0, +1}
  bw  = scaling[co] * sign(conv_w)             scaling = mean |conv_w| per out-ch
  y   = conv2d(act, bw, pad=1)
  y   = prelu(y + pr_bias0) + pr_bias1 + x     prelu slope per channel

Strategy: data-parallel over batch, 2 images per core (8 cores).  The two
images' 64 channels are stacked on the 128 SBUF partitions.  x is shipped to
the device in bf16 (halves HBM reads); y is produced in bf16 and upcast on
the host (halves HBM writes).  Activations are binarized to fp8 +-1 on the
Scalar engine.  The 3x3 conv runs as fp8 DoubleRow matmuls with
block-diagonal +-1 weights (exact integer arithmetic in fp32 PSUM):

  - taps (kh=0,kw)+(kh=1,kw) pair along the act row stride (3 DR matmuls
    per output row),
  - taps (kh=2,kw=0)+(kh=2,kw=1) pair across a column-shifted copy of the
    act plane (delta stride = plane pitch, 16B-aligned as DR requires); the
    shifted plane is produced by a cheap SBUF->SBUF DMA,
  - tap (kh=2,kw=2) is a plain matmul over a 2-row pair.

That is 5 PE streaming cycles per output column (vs 6 for the kh-pair-only
scheme); the 9-tap/DoubleRow parity floor is 4.5.

PSUM is organized as two 8-row tiles (4 banks each) so the per-channel
scale/bias PSUM->SBUF drain is one ACT instruction per 8 rows, and the
PReLU + residual run as two bf16 ops per 8 rows on the Vector engine.

pr_bias1 is folded into x on the host (x' = x + b1, sign bias b' = b - k*b1)
so the residual step is a plain tensor_tensor add (2x DVE rate for bf16)
instead of a scalar_tensor_tensor.
"""

import sys

if "/opt/trn_rl_repo" not in sys.path:
    sys.path.insert(0, "/opt/trn_rl_repo")

from contextlib import ExitStack

import ml_dtypes
import numpy as np

import concourse.bacc as bacc
import concourse.bass as bass
import concourse.tile as tile
from concourse import mybir
from concourse.bass_utils import run_bass_kernel_spmd

B, C, H, W = 16, 64, 256, 256
NCORES = 8
HS = 32                      # output rows per strip
P = 128                      # partitions = 2 images x 64 channels

F32 = mybir.dt.float32
BF16 = mybir.dt.bfloat16
FP8 = mybir.dt.float8e4
AF = mybir.ActivationFunctionType
ALU = mybir.AluOpType
DR = mybir.MatmulPerfMode.DoubleRow

APITCH = 272                 # act row pitch (bytes %16 for DoubleRow AP steps)
GROUP = 8                    # output rows per PSUM tile (4 banks)

# Param table columns (per-partition f32 scalars)
PK, PB, PS, PB0, PCM, PB1, PSL = 0, 1, 2, 3, 4, 5, 6

# prelu(u) == max(u, slope*u) when 0 <= slope <= 1 (checked at runtime in
# kernel()); one fused scalar_tensor_tensor op instead of tensor_scalar +
# tensor_tensor.  Set False for the general min/mult path.
PRELU_MAX_TRICK = True
USE_LRELU = False            # kept for test.py compat

SIGN_CHUNK = 9               # rows of sign-activation per ACT instruction
STRIP_HS = [32] * 8          # strip heights (sum == H)


def _emit(tc, nc, x_d, w_d, p_d, pb_d, y_d):
    x3 = x_d.rearrange("p (h w) -> p h w", w=W)
    y3 = y_d.rearrange("p (h w) -> p h w", w=W)

    with ExitStack() as ctx:
        consts = ctx.enter_context(tc.tile_pool(name="consts", bufs=1))
        xpool = ctx.enter_context(tc.tile_pool(name="xpool", bufs=3))
        apool = ctx.enter_context(tc.tile_pool(name="apool", bufs=3))
        ypool = ctx.enter_context(tc.tile_pool(name="ypool", bufs=2))
        vpool = ctx.enter_context(tc.tile_pool(name="vpool", bufs=4))
        pspool = ctx.enter_context(tc.tile_pool(name="pspool", bufs=2,
                                                space="PSUM"))

        # a throwaway 1-element activation forces the ACT_TABLE_LOAD to run
        # concurrently with the input DMAs instead of blocking the first sign
        dummy = consts.tile([P, 1], F32)
        wdum = consts.tile([P, 128], FP8)   # zero weights for the PE warmup
        nc.gpsimd.memset(wdum, 0.0)
        nc.gpsimd.memset(dummy, 0.0)
        nc.scalar.activation(dummy, dummy, AF.Sign, bias=0.0, scale=1.0)

        # spread the startup DMAs over all three rings so the first sign
        # (needs pt + x rows 0-5) is not serialized behind anything else:
        # x alone on the sync ring, params on the scalar ring (the act-shift
        # copy ring, idle at start), weights + bf16 params on gpsimd
        pt = consts.tile([P, 8], F32)
        nc.scalar.dma_start(out=pt, in_=p_d)
        # [kw, delta(kh 0/1), m] DoubleRow weights for the kh={0,1} pairs
        wdr = consts.tile([P, 3, 2, 128], FP8)
        nc.gpsimd.dma_start(out=wdr, in_=w_d[:, :768].rearrange(
            "p (k d m) -> p k d m", k=3, d=2))
        # [delta(kw 0/1), m] DoubleRow weights for the kh=2 kw-pair
        w2 = consts.tile([P, 2, 128], FP8)
        nc.gpsimd.dma_start(out=w2, in_=w_d[:, 768:1024].rearrange(
            "p (d m) -> p d m", d=2))
        # [m] plain weights for the lone (kh=2,kw=2) tap
        wn = consts.tile([P, 128], FP8)
        nc.gpsimd.dma_start(out=wn, in_=w_d[:, 1024:1152])
        ptb = consts.tile([P, 8], BF16)   # bf16 scalars for the DVE stt
        nc.gpsimd.dma_start(out=ptb, in_=pb_d)

        # PE p-state warmup: throwaway matmuls keep the PE streaming from
        # the moment the engine preamble ends, so the DVFS ramp (low -> mid
        # -> full over ~3us of continuous execution) completes before the
        # first real matmul instead of slowing the whole first strip ~1.5x
        ps_w = pspool.tile([P, GROUP, W], F32, name="ps")
        for _ in range(36):
            nc.tensor.matmul(ps_w[:, 0, 0:128], lhsT=wdum, rhs=wdum,
                             start=True, stop=True)

        H0S = [sum(STRIP_HS[:i]) for i in range(len(STRIP_HS))]
        NST = len(STRIP_HS)
        HSMAX = max(STRIP_HS)

        def strip_rows(s):
            h0 = H0S[s]
            row_lo = max(h0 - 1, 0)
            row_hi = min(h0 + STRIP_HS[s] + 1, H)
            return h0, row_lo, row_hi, row_lo - (h0 - 1)

        def load_x(s):
            """DMA the x strip (rows h0-1 .. h0+hs; tile row a <-> global
            h0-1+a).  Issued two strips ahead of use."""
            h0, row_lo, row_hi, r0 = strip_rows(s)
            nr = row_hi - row_lo
            xs = xpool.tile([P, HSMAX + 2, W], BF16, name="xs")
            if s == 0:        # tiny first chunks: the first sign unblocks asap
                bounds = [row_lo + o for o in (0, 4, 10, 17, 24, nr)]
            else:
                bounds = [row_lo, row_lo + nr // 2, row_hi]
            for a, b in zip(bounds, bounds[1:]):
                if b > a:
                    nc.sync.dma_start(out=xs[:, a - (h0 - 1):b - (h0 - 1), :],
                                      in_=x3[:, a:b, :])
            return xs

        def prep_act(s):
            """act planes: [plane, row, col]; plane 1 is the +1-column
            shift; zero the padding."""
            act = apool.tile([P, 2, HSMAX + 2, APITCH], FP8, name="act")
            nrows = STRIP_HS[s] + 2
            nc.gpsimd.memset(act[:, 0, :nrows, 0:1], 0.0)
            nc.gpsimd.memset(act[:, 0, :nrows, W + 1:W + 2], 0.0)
            if s == 0:
                nc.gpsimd.memset(act[:, :, 0:1, :], 0.0)
            if s == NST - 1:
                nc.gpsimd.memset(act[:, :, nrows - 1:nrows, :], 0.0)
            return act

        def sign_strip(s, xs, act, chunks, skip=0, copy_after=None):
            """Binarize x into the zero-padded act plane 0, in row chunks
            (the first small so dependent matmuls unblock quickly), then
            DMA plane 1 = plane 0 shifted left one column.  copy_after
            selects the chunk indices after which the (latency-bound)
            shift copies are issued; default after every chunk."""
            _, row_lo, row_hi, r0 = strip_rows(s)
            c0 = r0 + skip
            pend = c0      # first row not yet shift-copied
            for idx, sz in enumerate(chunks):
                c1 = min(c0 + sz, r0 + (row_hi - row_lo))
                if c1 <= c0:
                    break
                nc.scalar.activation(
                    act[:, 0, c0:c1, 1:W + 1], xs[:, c0:c1, :], AF.Sign,
                    bias=pt[:, PB:PB + 1], scale=pt[:, PK:PK + 1],
                )
                c0 = c1
                # the shift copies issue from the gpsimd ring, which only
                # carries memsets and startup weights: they never queue
                # behind loads or stores on the other rings
                if copy_after is None or idx in copy_after:
                    nc.gpsimd.dma_start(out=act[:, 1, pend:c1, 0:W + 1],
                                        in_=act[:, 0, pend:c1, 1:W + 2])
                    pend = c1
            if pend < c0:
                nc.gpsimd.dma_start(out=act[:, 1, pend:c0, 0:W + 1],
                                    in_=act[:, 0, pend:c0, 1:W + 2])

        FIRST_CHUNKS = (3, 3, 4, 4, 5, 7, 7)   # strip 0: progressive chunks
        NEXT_CHUNKS = (5, 9, 9, 11)      # next strips: one chunk per group

        xs_tiles = {0: load_x(0)}
        act_cur = prep_act(0)
        # strip 0: two coarse shift copies (rows 1-15, 15-34) timed to land
        # just before groups 1 and 2 need them; group 0 needs none (its kh=2
        # taps run plain).  Seven per-chunk copies serialize ~2us each on
        # the ring and arrive too late.
        sign_strip(0, xs_tiles[0], act_cur, FIRST_CHUNKS, copy_after={3, 6})
        xs_tiles[1] = load_x(1)
        act_nxt = None
        for s in range(NST):
            h0 = H0S[s]
            HS_S = STRIP_HS[s]
            NG = HS_S // GROUP
            xs, act = xs_tiles.pop(s), act_cur
            if s + 2 < NST:
                xs_tiles[s + 2] = load_x(s + 2)  # two strips of load lookahead
            if s + 1 < NST:
                act_nxt = prep_act(s + 1)
            ys = ypool.tile([P, HSMAX, W], BF16, name="ys")
            for g in range(NG):
                # the next strip's binarization leads this strip's PSUM
                # drains on the ACT engine: chunk g runs while the PE is
                # still streaming group g, so the sign -> shift-copy chain
                # completes a full strip before its kh=2 matmuls need it
                if s + 1 < NST:
                    sign_strip(s + 1, xs_tiles[s + 1], act_nxt,
                               NEXT_CHUNKS[g:g + 1],
                               skip=sum(NEXT_CHUNKS[:g]))
                ps = pspool.tile([P, GROUP, W], F32, name="ps")
                # all kh={0,1} taps (plane 0 only, gated by sign) run first;
                # the kh=2 taps that need the shifted plane-1 copy run last,
                # giving the sign->copy chain ~4us of PE slack per group
                for j in range(GROUP // 2):
                    rr = GROUP * g + 2 * j   # strip-local first row of pair
                    for kw in range(3):
                        for i in range(2):
                            # kh in {0,1} via DoubleRow: contraction over
                            # (partition, delta), act rows (rr+i)+{0,1}
                            nc.tensor.matmul(
                                ps[:, 2 * j + i, :],
                                lhsT=wdr[:, kw, :, :],
                                rhs=act[:, 0, rr + i:rr + i + 2, kw:kw + W],
                                start=(kw == 0 and i == 0),
                                stop=False,
                                perf_mode=DR,
                            )
                for j in range(GROUP // 2):
                    rr = GROUP * g + 2 * j
                    if g == 0 and s <= 1:
                        # the first group of the first two strips cannot
                        # afford to wait for the sign->shift-copy chain: its
                        # kh=2 kw={0,1} taps run as plain matmuls straight
                        # off plane 0 (+0.5us PE per group, but removes the
                        # startup copy latency entirely)
                        for kw in range(2):
                            nc.tensor.matmul(
                                ps[:, 2 * j:2 * j + 2, :],
                                lhsT=w2[:, kw, :],
                                rhs=act[:, 0, rr + 2:rr + 4, kw:kw + W],
                                start=False,
                                stop=False,
                            )
                    else:
                        for i in range(2):
                            # kh=2, kw in {0,1} via DoubleRow across the two
                            # act planes (plane 1 = plane 0 shifted 1 column)
                            nc.tensor.matmul(
                                ps[:, 2 * j + i, :],
                                lhsT=w2,
                                rhs=act[:, 0:2, rr + i + 2, 0:W],
                                start=False,
                                stop=False,
                                perf_mode=DR,
                            )
                    # lone (kh=2,kw=2) tap: plain matmul over both rows
                    nc.tensor.matmul(
                        ps[:, 2 * j:2 * j + 2, :],
                        lhsT=wn,
                        rhs=act[:, 0, rr + 2:rr + 4, 2:2 + W],
                        start=False,
                        stop=True,
                    )
                def drain(ps_rows, r0, nr):
                    # v = ps*scaling + b0 (PSUM->SBUF on ACT), then PReLU +
                    # residual on DVE, then store (sync ring, behind the
                    # prefetched loads)
                    v = vpool.tile([P, GROUP, W], BF16, name="v")[:, :nr, :]
                    nc.scalar.activation(
                        v, ps_rows, AF.Identity,
                        bias=pt[:, PB0:PB0 + 1], scale=pt[:, PS:PS + 1],
                    )
                    u = ys[:, r0:r0 + nr, :]
                    xres = xs[:, r0 + 1:r0 + 1 + nr, :]
                    if PRELU_MAX_TRICK:
                        nc.vector.scalar_tensor_tensor(
                            u, v, ptb[:, PSL:PSL + 1], v, ALU.mult, ALU.max
                        )
                        nc.vector.tensor_tensor(u, xres, u, ALU.add)
                    else:
                        # u = v + (slope-1)*min(v, 0); u += x' (= x + b1)
                        m = vpool.tile([P, GROUP, W], BF16,
                                       name="m")[:, :nr, :]
                        nc.vector.tensor_scalar(
                            m, v, 0.0, pt[:, PCM:PCM + 1], ALU.min, ALU.mult
                        )
                        nc.vector.tensor_tensor(u, v, m, ALU.add)
                        nc.vector.tensor_tensor(u, xres, u, ALU.add)
                    nc.sync.dma_start(out=y3[:, h0 + r0:h0 + r0 + nr, :],
                                      in_=ys[:, r0:r0 + nr, :])

                if s == NST - 1 and g == NG - 1:
                    # final group drains in shrinking pieces so the
                    # post-op/store chain after the last matmul is as short
                    # as possible
                    drain(ps[:, 0:4, :], GROUP * g, 4)
                    drain(ps[:, 4:6, :], GROUP * g + 4, 2)
                    drain(ps[:, 6:8, :], GROUP * g + 6, 2)
                else:
                    drain(ps, GROUP * g, GROUP)
            act_cur = act_nxt


def build_nc():
    nc = bacc.Bacc("TRN2", target_bir_lowering=False, debug=False,
                   num_devices=NCORES)
    x_d = nc.dram_tensor("xin", [P, H * W], BF16, kind="ExternalInput").ap()
    w_d = nc.dram_tensor("wp", [P, 9 * 128], FP8, kind="ExternalInput").ap()
    p_d = nc.dram_tensor("pp", [P, 8], F32, kind="ExternalInput").ap()
    pb_d = nc.dram_tensor("ppb", [P, 8], BF16, kind="ExternalInput").ap()
    y_d = nc.dram_tensor("yout", [P, H * W], BF16, kind="ExternalOutput").ap()
    with tile.TileContext(nc) as tc:
        _emit(tc, nc, x_d, w_d, p_d, pb_d, y_d)
    nc.compile()
    return nc


_NC_CACHE = {}


def _get_nc():
    key = (PRELU_MAX_TRICK,)
    if key not in _NC_CACHE:
        _NC_CACHE[key] = build_nc()
    return _NC_CACHE[key]


def make_inputs(x, rd_k, rd_b, beta, conv_w, pr_bias0, prelu_w, pr_bias1):
    """Host-side prep: per-channel param table, packed sign weights, shards."""
    k = np.asarray(rd_k, np.float32).reshape(C)
    b = np.asarray(rd_b, np.float32).reshape(C)
    s = np.mean(np.abs(np.asarray(conv_w, np.float32)), axis=(1, 2, 3))
    b0 = np.asarray(pr_bias0, np.float32).reshape(C)
    slope = np.asarray(prelu_w, np.float32).reshape(C)
    b1 = np.asarray(pr_bias1, np.float32).reshape(C)
    cm = slope - 1.0
    # b1 is folded into x (x' = x + b1); the sign threshold compensates
    bs = b - k * b1
    cols = np.stack([k, bs, s, b0, cm, b1, slope, np.zeros(C, np.float32)],
                    axis=1)
    pp = np.concatenate([cols, cols], axis=0).astype(np.float32)  # [128, 8]
    ppb = pp.astype(ml_dtypes.bfloat16)

    sw = np.sign(np.asarray(conv_w, np.float32)).astype(np.float32)

    def blockdiag(kh, kw):
        S = sw[:, :, kh, kw].T  # [ci, co]
        out = np.zeros((P, P), np.float32)
        out[0:C, 0:C] = S
        out[C:P, C:P] = S
        return out

    wp = np.zeros((P, 9, 128), np.float32)
    for kw in range(3):            # [kw, delta(kh 0/1), m] DoubleRow pairs
        for d in range(2):
            wp[:, kw * 2 + d, :] = blockdiag(d, kw)
    wp[:, 6, :] = blockdiag(2, 0)  # [delta(kw 0/1), m] kh=2 DR pair
    wp[:, 7, :] = blockdiag(2, 1)
    wp[:, 8, :] = blockdiag(2, 2)  # lone (kh=2,kw=2)
    wdt = mybir.dt.np(FP8)
    wp = np.ascontiguousarray(wp.reshape(P, 9 * 128)).astype(wdt)

    x = np.asarray(x, np.float32) + b1[None, :, None, None]
    x = x.astype(ml_dtypes.bfloat16)
    in_maps = []
    for c in range(NCORES):
        xc = np.ascontiguousarray(x[2 * c:2 * c + 2]).reshape(P, H * W)
        in_maps.append({"xin": xc, "wp": wp, "pp": pp, "ppb": ppb})
    return in_maps


def kernel(x, rd_k, rd_b, beta, conv_w, pr_bias0, prelu_w, pr_bias1):
    global PRELU_MAX_TRICK
    slope = np.asarray(prelu_w, np.float32).reshape(C)
    if not np.all((slope >= 0.0) & (slope <= 1.0)):
        PRELU_MAX_TRICK = False   # max-identity only valid for slope in [0,1]
    in_maps = make_inputs(x, rd_k, rd_b, beta, conv_w, pr_bias0, prelu_w,
                          pr_bias1)
    nc = _get_nc()
    res = run_bass_kernel_spmd(nc, in_maps, core_ids=list(range(NCORES)))
    y = np.empty((B, C, H, W), np.float32)
    for c in range(NCORES):
        y[2 * c:2 * c + 2] = np.asarray(
            res.results[c]["yout"], dtype=np.float32).reshape(2, C, H, W)
    return y


# revision 36
# speedup vs baseline: 1.0484x; 1.0484x over previous
"""Trainium2 Bass kernel for nn_BinaryConv2d (B=16, C=64, H=W=256, 3x3, pad 1).

Forward semantics (STE forward values):
  act = sign(x * rd_k + rd_b)                  in {-1, 0, +1}
  bw  = scaling[co] * sign(conv_w)             scaling = mean |conv_w| per out-ch
  y   = conv2d(act, bw, pad=1)
  y   = prelu(y + pr_bias0) + pr_bias1 + x     prelu slope per channel

Strategy: data-parallel over batch, 2 images per core (8 cores).  The two
images' 64 channels are stacked on the 128 SBUF partitions.  x is shipped to
the device in bf16 (halves HBM reads); y is produced in bf16 and upcast on
the host (halves HBM writes).  Activations are binarized to fp8 +-1 on the
Scalar engine.  The 3x3 conv runs as fp8 DoubleRow matmuls with
block-diagonal +-1 weights (exact integer arithmetic in fp32 PSUM):

  - taps (kh=0,kw)+(kh=1,kw) pair along the act row stride (3 DR matmuls
    per output row),
  - taps (kh=2,kw=0)+(kh=2,kw=1) pair across a column-shifted copy of the
    act plane (delta stride = plane pitch, 16B-aligned as DR requires); the
    shifted plane is produced by a cheap SBUF->SBUF DMA,
  - tap (kh=2,kw=2) is a plain matmul over a 2-row pair.

That is 5 PE streaming cycles per output column (vs 6 for the kh-pair-only
scheme); the 9-tap/DoubleRow parity floor is 4.5.

PSUM is organized as two 8-row tiles (4 banks each) so the per-channel
scale/bias PSUM->SBUF drain is one ACT instruction per 8 rows, and the
PReLU + residual run as two bf16 ops per 8 rows on the Vector engine.

pr_bias1 is folded into x on the host (x' = x + b1, sign bias b' = b - k*b1)
so the residual step is a plain tensor_tensor add (2x DVE rate for bf16)
instead of a scalar_tensor_tensor.
"""

import sys

if "/opt/trn_rl_repo" not in sys.path:
    sys.path.insert(0, "/opt/trn_rl_repo")

from contextlib import ExitStack

import ml_dtypes
import numpy as np

import concourse.bacc as bacc
import concourse.bass as bass
import concourse.tile as tile
from concourse import mybir
from concourse.bass_utils import run_bass_kernel_spmd

B, C, H, W = 16, 64, 256, 256
NCORES = 8
HS = 32                      # output rows per strip
P = 128                      # partitions = 2 images x 64 channels

F32 = mybir.dt.float32
BF16 = mybir.dt.bfloat16
FP8 = mybir.dt.float8e4
AF = mybir.ActivationFunctionType
ALU = mybir.AluOpType
DR = mybir.MatmulPerfMode.DoubleRow

APITCH = 272                 # act row pitch (bytes %16 for DoubleRow AP steps)
GROUP = 8                    # output rows per PSUM tile (4 banks)

# Param table columns (per-partition f32 scalars)
PK, PB, PS, PB0, PCM, PB1, PSL = 0, 1, 2, 3, 4, 5, 6

# prelu(u) == max(u, slope*u) when 0 <= slope <= 1 (checked at runtime in
# kernel()); one fused scalar_tensor_tensor op instead of tensor_scalar +
# tensor_tensor.  Set False for the general min/mult path.
PRELU_MAX_TRICK = True
USE_LRELU = False            # kept for test.py compat

SIGN_CHUNK = 9               # rows of sign-activation per ACT instruction
STRIP_HS = [32] * 8          # strip heights (sum == H)


def _emit(tc, nc, x_d, w_d, p_d, pb_d, y_d):
    x3 = x_d.rearrange("p (h w) -> p h w", w=W)
    y3 = y_d.rearrange("p (h w) -> p h w", w=W)

    with ExitStack() as ctx:
        consts = ctx.enter_context(tc.tile_pool(name="consts", bufs=1))
        xpool = ctx.enter_context(tc.tile_pool(name="xpool", bufs=3))
        apool = ctx.enter_context(tc.tile_pool(name="apool", bufs=3))
        ypool = ctx.enter_context(tc.tile_pool(name="ypool", bufs=2))
        vpool = ctx.enter_context(tc.tile_pool(name="vpool", bufs=4))
        pspool = ctx.enter_context(tc.tile_pool(name="pspool", bufs=2,
                                                space="PSUM"))

        # a throwaway 1-element activation forces the ACT_TABLE_LOAD to run
        # concurrently with the input DMAs instead of blocking the first sign
        dummy = consts.tile([P, 1], F32)
        wdum = consts.tile([P, 128], FP8)   # zero weights for the PE warmup
        nc.gpsimd.memset(wdum, 0.0)
        nc.gpsimd.memset(dummy, 0.0)
        nc.scalar.activation(dummy, dummy, AF.Sign, bias=0.0, scale=1.0)

        # spread the startup DMAs over all three rings so the first sign
        # (needs pt + x rows 0-5) is not serialized behind anything else:
        # x alone on the sync ring, params on the scalar ring (the act-shift
        # copy ring, idle at start), weights + bf16 params on gpsimd
        pt = consts.tile([P, 8], F32)
        nc.scalar.dma_start(out=pt, in_=p_d)
        # [kw, delta(kh 0/1), m] DoubleRow weights for the kh={0,1} pairs
        wdr = consts.tile([P, 3, 2, 128], FP8)
        nc.gpsimd.dma_start(out=wdr, in_=w_d[:, :768].rearrange(
            "p (k d m) -> p k d m", k=3, d=2))
        # [delta(kw 0/1), m] DoubleRow weights for the kh=2 kw-pair
        w2 = consts.tile([P, 2, 128], FP8)
        nc.gpsimd.dma_start(out=w2, in_=w_d[:, 768:1024].rearrange(
            "p (d m) -> p d m", d=2))
        # [m] plain weights for the lone (kh=2,kw=2) tap
        wn = consts.tile([P, 128], FP8)
        nc.gpsimd.dma_start(out=wn, in_=w_d[:, 1024:1152])
        ptb = consts.tile([P, 8], BF16)   # bf16 scalars for the DVE stt
        nc.gpsimd.dma_start(out=ptb, in_=pb_d)

        # PE p-state warmup: throwaway matmuls keep the PE streaming from
        # the moment the engine preamble ends, so the DVFS ramp (low -> mid
        # -> full over ~3us of continuous execution) completes before the
        # first real matmul instead of slowing the whole first strip ~1.5x
        ps_w = pspool.tile([P, GROUP, W], F32, name="ps")
        for _ in range(36):
            nc.tensor.matmul(ps_w[:, 0, 0:128], lhsT=wdum, rhs=wdum,
                             start=True, stop=True)

        H0S = [sum(STRIP_HS[:i]) for i in range(len(STRIP_HS))]
        NST = len(STRIP_HS)
        HSMAX = max(STRIP_HS)

        def strip_rows(s):
            h0 = H0S[s]
            row_lo = max(h0 - 1, 0)
            row_hi = min(h0 + STRIP_HS[s] + 1, H)
            return h0, row_lo, row_hi, row_lo - (h0 - 1)

        def load_x(s):
            """DMA the x strip (rows h0-1 .. h0+hs; tile row a <-> global
            h0-1+a).  Issued two strips ahead of use."""
            h0, row_lo, row_hi, r0 = strip_rows(s)
            nr = row_hi - row_lo
            xs = xpool.tile([P, HSMAX + 2, W], BF16, name="xs")
            if s == 0:        # tiny first chunks: the first sign unblocks asap
                bounds = [row_lo + o for o in (0, 4, 10, 17, 24, nr)]
            else:
                bounds = [row_lo, row_lo + nr // 2, row_hi]
            for a, b in zip(bounds, bounds[1:]):
                if b > a:
                    nc.sync.dma_start(out=xs[:, a - (h0 - 1):b - (h0 - 1), :],
                                      in_=x3[:, a:b, :])
            return xs

        def prep_act(s):
            """act planes: [plane, row, col]; plane 1 is the +1-column
            shift; zero the padding."""
            act = apool.tile([P, 2, HSMAX + 2, APITCH], FP8, name="act")
            nrows = STRIP_HS[s] + 2
            nc.gpsimd.memset(act[:, 0, :nrows, 0:1], 0.0)
            nc.gpsimd.memset(act[:, 0, :nrows, W + 1:W + 2], 0.0)
            if s == 0:
                nc.gpsimd.memset(act[:, :, 0:1, :], 0.0)
            if s == NST - 1:
                nc.gpsimd.memset(act[:, :, nrows - 1:nrows, :], 0.0)
            return act

        def sign_strip(s, xs, act, chunks, skip=0, copy_after=None):
            """Binarize x into the zero-padded act plane 0, in row chunks
            (the first small so dependent matmuls unblock quickly), then
            DMA plane 1 = plane 0 shifted left one column.  copy_after
            selects the chunk indices after which the (latency-bound)
            shift copies are issued; default after every chunk."""
            _, row_lo, row_hi, r0 = strip_rows(s)
            c0 = r0 + skip
            pend = c0      # first row not yet shift-copied
            for idx, sz in enumerate(chunks):
                c1 = min(c0 + sz, r0 + (row_hi - row_lo))
                if c1 <= c0:
                    break
                nc.scalar.activation(
                    act[:, 0, c0:c1, 1:W + 1], xs[:, c0:c1, :], AF.Sign,
                    bias=pt[:, PB:PB + 1], scale=pt[:, PK:PK + 1],
                )
                c0 = c1
                # the shift copies issue from the gpsimd ring, which only
                # carries memsets and startup weights: they never queue
                # behind loads or stores on the other rings
                if copy_after is None or idx in copy_after:
                    nc.gpsimd.dma_start(out=act[:, 1, pend:c1, 0:W + 1],
                                        in_=act[:, 0, pend:c1, 1:W + 2])
                    pend = c1
            if pend < c0 and copy_after is None:
                nc.gpsimd.dma_start(out=act[:, 1, pend:c0, 0:W + 1],
                                    in_=act[:, 0, pend:c0, 1:W + 2])

        FIRST_CHUNKS = (3, 3, 4, 4, 5, 7, 7)   # strip 0: progressive chunks
        NEXT_CHUNKS = (5, 9, 9, 11)      # next strips: one chunk per group

        xs_tiles = {0: load_x(0)}
        act_cur = prep_act(0)
        # strip 0 runs all kh=2 taps as plain matmuls off plane 0 (the
        # SBUF->SBUF shift copy is too slow to make the startup deadline),
        # so it needs no shift copies at all
        sign_strip(0, xs_tiles[0], act_cur, FIRST_CHUNKS, copy_after=())
        xs_tiles[1] = load_x(1)
        act_nxt = None
        for s in range(NST):
            h0 = H0S[s]
            HS_S = STRIP_HS[s]
            NG = HS_S // GROUP
            xs, act = xs_tiles.pop(s), act_cur
            if s + 2 < NST:
                xs_tiles[s + 2] = load_x(s + 2)  # two strips of load lookahead
            if s + 1 < NST:
                act_nxt = prep_act(s + 1)
            ys = ypool.tile([P, HSMAX, W], BF16, name="ys")
            for g in range(NG):
                # the next strip's binarization leads this strip's PSUM
                # drains on the ACT engine: chunk g runs while the PE is
                # still streaming group g, so the sign -> shift-copy chain
                # completes a full strip before its kh=2 matmuls need it
                if s + 1 < NST:
                    sign_strip(s + 1, xs_tiles[s + 1], act_nxt,
                               NEXT_CHUNKS[g:g + 1],
                               skip=sum(NEXT_CHUNKS[:g]))
                ps = pspool.tile([P, GROUP, W], F32, name="ps")
                # all kh={0,1} taps (plane 0 only, gated by sign) run first;
                # the kh=2 taps that need the shifted plane-1 copy run last,
                # giving the sign->copy chain ~4us of PE slack per group
                for j in range(GROUP // 2):
                    rr = GROUP * g + 2 * j   # strip-local first row of pair
                    for kw in range(3):
                        for i in range(2):
                            # kh in {0,1} via DoubleRow: contraction over
                            # (partition, delta), act rows (rr+i)+{0,1}
                            nc.tensor.matmul(
                                ps[:, 2 * j + i, :],
                                lhsT=wdr[:, kw, :, :],
                                rhs=act[:, 0, rr + i:rr + i + 2, kw:kw + W],
                                start=(kw == 0 and i == 0),
                                stop=False,
                                perf_mode=DR,
                            )
                for j in range(GROUP // 2):
                    rr = GROUP * g + 2 * j
                    if s == 0 or (s == 1 and g == 0):
                        # strip 0 (and strip 1's first group) cannot afford
                        # to wait for the sign->shift-copy chain: their kh=2
                        # kw={0,1} taps run as plain matmuls straight off
                        # plane 0 (+260ns PE per row pair, but no startup
                        # copy latency)
                        for kw in range(2):
                            nc.tensor.matmul(
                                ps[:, 2 * j:2 * j + 2, :],
                                lhsT=w2[:, kw, :],
                                rhs=act[:, 0, rr + 2:rr + 4, kw:kw + W],
                                start=False,
                                stop=False,
                            )
                    else:
                        for i in range(2):
                            # kh=2, kw in {0,1} via DoubleRow across the two
                            # act planes (plane 1 = plane 0 shifted 1 column)
                            nc.tensor.matmul(
                                ps[:, 2 * j + i, :],
                                lhsT=w2,
                                rhs=act[:, 0:2, rr + i + 2, 0:W],
                                start=False,
                                stop=False,
                                perf_mode=DR,
                            )
                    # lone (kh=2,kw=2) tap: plain matmul over both rows
                    nc.tensor.matmul(
                        ps[:, 2 * j:2 * j + 2, :],
                        lhsT=wn,
                        rhs=act[:, 0, rr + 2:rr + 4, 2:2 + W],
                        start=False,
                        stop=True,
                    )
                def drain(ps_rows, r0, nr):
                    # v = ps*scaling + b0 (PSUM->SBUF on ACT), then PReLU +
                    # residual on DVE, then store (sync ring, behind the
                    # prefetched loads)
                    v = vpool.tile([P, GROUP, W], BF16, name="v")[:, :nr, :]
                    nc.scalar.activation(
                        v, ps_rows, AF.Identity,
                        bias=pt[:, PB0:PB0 + 1], scale=pt[:, PS:PS + 1],
                    )
                    u = ys[:, r0:r0 + nr, :]
                    xres = xs[:, r0 + 1:r0 + 1 + nr, :]
                    if PRELU_MAX_TRICK:
                        nc.vector.scalar_tensor_tensor(
                            u, v, ptb[:, PSL:PSL + 1], v, ALU.mult, ALU.max
                        )
                        nc.vector.tensor_tensor(u, xres, u, ALU.add)
                    else:
                        # u = v + (slope-1)*min(v, 0); u += x' (= x + b1)
                        m = vpool.tile([P, GROUP, W], BF16,
                                       name="m")[:, :nr, :]
                        nc.vector.tensor_scalar(
                            m, v, 0.0, pt[:, PCM:PCM + 1], ALU.min, ALU.mult
                        )
                        nc.vector.tensor_tensor(u, v, m, ALU.add)
                        nc.vector.tensor_tensor(u, xres, u, ALU.add)
                    nc.sync.dma_start(out=y3[:, h0 + r0:h0 + r0 + nr, :],
                                      in_=ys[:, r0:r0 + nr, :])

                if s == NST - 1 and g == NG - 1:
                    # final group drains in shrinking pieces so the
                    # post-op/store chain after the last matmul is as short
                    # as possible
                    drain(ps[:, 0:4, :], GROUP * g, 4)
                    drain(ps[:, 4:6, :], GROUP * g + 4, 2)
                    drain(ps[:, 6:8, :], GROUP * g + 6, 2)
                else:
                    drain(ps, GROUP * g, GROUP)
            act_cur = act_nxt


def build_nc():
    nc = bacc.Bacc("TRN2", target_bir_lowering=False, debug=False,
                   num_devices=NCORES)
    x_d = nc.dram_tensor("xin", [P, H * W], BF16, kind="ExternalInput").ap()
    w_d = nc.dram_tensor("wp", [P, 9 * 128], FP8, kind="ExternalInput").ap()
    p_d = nc.dram_tensor("pp", [P, 8], F32, kind="ExternalInput").ap()
    pb_d = nc.dram_tensor("ppb", [P, 8], BF16, kind="ExternalInput").ap()
    y_d = nc.dram_tensor("yout", [P, H * W], BF16, kind="ExternalOutput").ap()
    with tile.TileContext(nc) as tc:
        _emit(tc, nc, x_d, w_d, p_d, pb_d, y_d)
    nc.compile()
    return nc


_NC_CACHE = {}


def _get_nc():
    key = (PRELU_MAX_TRICK,)
    if key not in _NC_CACHE:
        _NC_CACHE[key] = build_nc()
    return _NC_CACHE[key]


def make_inputs(x, rd_k, rd_b, beta, conv_w, pr_bias0, prelu_w, pr_bias1):
    """Host-side prep: per-channel param table, packed sign weights, shards."""
    k = np.asarray(rd_k, np.float32).reshape(C)
    b = np.asarray(rd_b, np.float32).reshape(C)
    s = np.mean(np.abs(np.asarray(conv_w, np.float32)), axis=(1, 2, 3))
    b0 = np.asarray(pr_bias0, np.float32).reshape(C)
    slope = np.asarray(prelu_w, np.float32).reshape(C)
    b1 = np.asarray(pr_bias1, np.float32).reshape(C)
    cm = slope - 1.0
    # b1 is folded into x (x' = x + b1); the sign threshold compensates
    bs = b - k * b1
    cols = np.stack([k, bs, s, b0, cm, b1, slope, np.zeros(C, np.float32)],
                    axis=1)
    pp = np.concatenate([cols, cols], axis=0).astype(np.float32)  # [128, 8]
    ppb = pp.astype(ml_dtypes.bfloat16)

    sw = np.sign(np.asarray(conv_w, np.float32)).astype(np.float32)

    def blockdiag(kh, kw):
        S = sw[:, :, kh, kw].T  # [ci, co]
        out = np.zeros((P, P), np.float32)
        out[0:C, 0:C] = S
        out[C:P, C:P] = S
        return out

    wp = np.zeros((P, 9, 128), np.float32)
    for kw in range(3):            # [kw, delta(kh 0/1), m] DoubleRow pairs
        for d in range(2):
            wp[:, kw * 2 + d, :] = blockdiag(d, kw)
    wp[:, 6, :] = blockdiag(2, 0)  # [delta(kw 0/1), m] kh=2 DR pair
    wp[:, 7, :] = blockdiag(2, 1)
    wp[:, 8, :] = blockdiag(2, 2)  # lone (kh=2,kw=2)
    wdt = mybir.dt.np(FP8)
    wp = np.ascontiguousarray(wp.reshape(P, 9 * 128)).astype(wdt)

    x = np.asarray(x, np.float32) + b1[None, :, None, None]
    x = x.astype(ml_dtypes.bfloat16)
    in_maps = []
    for c in range(NCORES):
        xc = np.ascontiguousarray(x[2 * c:2 * c + 2]).reshape(P, H * W)
        in_maps.append({"xin": xc, "wp": wp, "pp": pp, "ppb": ppb})
    return in_maps


def kernel(x, rd_k, rd_b, beta, conv_w, pr_bias0, prelu_w, pr_bias1):
    global PRELU_MAX_TRICK
    slope = np.asarray(prelu_w, np.float32).reshape(C)
    if not np.all((slope >= 0.0) & (slope <= 1.0)):
        PRELU_MAX_TRICK = False   # max-identity only valid for slope in [0,1]
    in_maps = make_inputs(x, rd_k, rd_b, beta, conv_w, pr_bias0, prelu_w,
                          pr_bias1)
    nc = _get_nc()
    res = run_bass_kernel_spmd(nc, in_maps, core_ids=list(range(NCORES)))
    y = np.empty((B, C, H, W), np.float32)
    for c in range(NCORES):
        y[2 * c:2 * c + 2] = np.asarray(
            res.results[c]["yout"], dtype=np.float32).reshape(2, C, H, W)
    return y
